# revision 8
# baseline (speedup 1.0000x reference)
"""Trainium2 Bass kernel for im2col Conv2d dot-product:
out[b, n] = <enc_x[b, n, :], w_flat> + bias.

Data-parallel over batch: 8 batches per NeuronCore x 8 cores.
Per core: x is [401408, 49] -> out [401408] fp32.

v2: TensorE matmul formulation (baseline v1 was DVE/GpSimd-bound at
344us: tensor_reduce is capped at 1x = ~160us alone, DMA only 61% busy).

  - Host repacks x to bf16, k-major pair layout: for window-row pair
    q (rows 2q, 2q+1 of the [128, 3136] output grid), xT[q] is
    [98, 3136] with partition c = 49*a + k, free j.  Halves HBM
    traffic (39.3 MB/core) and puts the contraction dim on partitions.
  - Stationary: 16 zero-padded block-diagonal weight tiles [98, 32]
    (r = q%16): col 2r+a carries w at rows 49a..49a+48.  A matmul
    lhsT=stat[:,32r:32r+32], rhs=xT[q] chunk writes out rows
    [32b, 32b+32) of PSUM (b = q//16; out partition base 32b is legal:
    32-aligned tile_position).  Multiply+reduce happen in the PE
    array; 448 matmuls x 512 cols ~ 96us, under the ~110us DMA floor.
  - Zero columns of the stationary write zeros; 16 r-matmuls per strip
    accumulate (start at r=0, stop at r=15), each contributing its 2
    real rows.
  - Strip close: ScalarE activation adds bias and copies PSUM->SBUF,
    gpsimd-queue DMA writes the strip; overlaps the next strip's
    matmul stream.  Tail after last matmul is ~4us.
"""

from contextlib import ExitStack

import numpy as np
import ml_dtypes

import concourse.bass as bass
import concourse.tile as tile
from concourse import mybir

B = 64
WINDOWS = 50176
K = 49
NCORES = 8
BPC = B // NCORES            # batches per core
NWIN = BPC * WINDOWS         # 401408 windows per core
ROWS = 128                   # window-row grid: NWIN = ROWS * J
J = NWIN // ROWS             # 3136
NPAIR = ROWS // 2            # 64 row pairs (q)
NSTAT = 16                   # stationaries (r = q % 16)
CHUNK = 512                  # matmul moving cols = one PSUM bank of fp32
G = 4                        # pairs loaded per DMA (16 q's per strip, G | 16)

FP32 = mybir.dt.float32
BF16 = mybir.dt.bfloat16
BF16_NP = ml_dtypes.bfloat16

_NC = None


def _build_nc():
    nc = bass.Bass(trn_type="TRN2", debug=False, num_devices=NCORES)

    x = nc.dram_tensor("x", [2 * K, NPAIR * J], BF16, kind="ExternalInput").ap()
    s = nc.dram_tensor("s", [2 * K, 32 * NSTAT], BF16, kind="ExternalInput").ap()
    b = nc.dram_tensor("b", [1], FP32, kind="ExternalInput").ap()
    out = nc.dram_tensor("out", [NWIN], FP32, kind="ExternalOutput").ap()

    with tile.TileContext(nc) as tc, ExitStack() as ctx:
        consts = ctx.enter_context(tc.tile_pool(name="consts", bufs=1))
        xpool = ctx.enter_context(tc.tile_pool(name="x", bufs=4))
        ppool = ctx.enter_context(tc.tile_pool(name="psum", bufs=1, space="PSUM"))

        stat = consts.tile([2 * K, 32 * NSTAT], BF16)
        nc.gpsimd.dma_start(out=stat[:], in_=s)
        bb = consts.tile([128, 1], FP32)
        nc.gpsimd.dma_start(
            out=bb[:],
            in_=bass.AP(tensor=b.tensor, offset=b.offset, ap=[[0, 128]] + list(b.ap)),
        )

        # Matmul out partition base is restricted to {0, 32, 64} (no 96):
        # psum holds 3 strips; strip 3 reuses psum base 0, whose strip-0
        # accumulation closed 32 pair-tiles earlier (no stall, no hazard).
        acc = ppool.tile([96, J], FP32)
        ot = consts.tile([96, J], FP32)

        # x is c-major: [98, NPAIR*J], so one DMA of G pairs moves
        # G*J*2 = 25 KB per partition descriptor (small descriptors ran
        # at ~17 GB/s/engine; ~25 KB runs near the ~28 GB/s/engine cap).
        for g in range(NPAIR // G):
            xt = xpool.tile([2 * K, G * J], BF16, tag="xt", name=f"xt{g}")
            src = bass.AP(
                tensor=x.tensor,
                offset=x.offset + g * G * J,
                ap=[[NPAIR * J, 2 * K], [1, G * J]],
            )
            nc.sync.dma_start(out=xt[:], in_=src)

            for gq in range(G):
                q = g * G + gq
                r, bstrip = q % NSTAT, q // NSTAT
                p0 = 32 * (bstrip % 3)
                for j0 in range(0, J, CHUNK):
                    j1 = min(J, j0 + CHUNK)
                    nc.tensor.matmul(
                        acc[p0 : p0 + 32, j0:j1],
                        stat[:, 32 * r : 32 * r + 32],
                        xt[:, gq * J + j0 : gq * J + j1],
                        start=(r == 0),
                        stop=(r == NSTAT - 1),
                    )

                if r == NSTAT - 1:
                    # strip complete: bias-add into SBUF, DMA out
                    nc.scalar.activation(
                        out=ot[p0 : p0 + 32, :],
                        in_=acc[p0 : p0 + 32, :],
                        func=mybir.ActivationFunctionType.Identity,
                        bias=bb[p0 : p0 + 32, 0:1],
                        scale=1.0,
                    )
                    dst = bass.AP(
                        tensor=out.tensor,
                        offset=out.offset + 32 * bstrip * J,
                        ap=[[J, 32], [1, J]],
                    )
                    nc.gpsimd.dma_start(out=dst, in_=ot[p0 : p0 + 32, :])

    return nc


def _split_ctrl_waits(nc, max_waits=1):
    """Work around a walrus codegen limit on this build: instructions accept
    only one sync-wait command. Hoist extra waits onto dedicated no-op
    instructions inserted just before, preserving per-engine order."""
    from concourse import mybir

    for f in nc.m.functions:
        for blk in f.blocks:
            insts = blk.instructions
            i = 0
            while i < len(insts):
                ins = insts[i]
                if (
                    ins.sync_info is not None
                    and len(ins.sync_info.on_wait) > max_waits
                ):
                    waits = list(ins.sync_info.on_wait)
                    keep, extra = waits[:max_waits], waits[max_waits:]
                    ins.sync_info.on_wait = keep
                    for j, wchunk in enumerate(extra):
                        nop = mybir.InstNoOp(
                            name=f"{ins.name}-wsplit{j}",
                            sync_info=mybir.SyncInfo(on_wait=[wchunk], on_update=[]),
                            bass_nofuse=True,
                            engine=ins.engine,
                        )
                        nc.register_instruction(nop, overwrite=True)
                        insts.insert(i, nop)
                        i += 1
                i += 1


def _get_nc():
    global _NC
    if _NC is None:
        _NC = _build_nc()
        _split_ctrl_waits(_NC)
    return _NC


def _pack_inputs(enc_x, weight, bias):
    """Host-side repack: bf16 k-major pair layout + stationary tiles."""
    # c-major: xT[c, 49*a + k, q*J + j] = enc_x_core_c[(2q+a)*J + j, k]
    xb = np.asarray(enc_x, dtype=np.float32).reshape(NCORES, NPAIR, 2, J, K)
    xT = xb.transpose(0, 2, 4, 1, 3).astype(BF16_NP)
    xT = np.ascontiguousarray(xT).reshape(NCORES, 2 * K, NPAIR * J)

    wb = np.asarray(weight, dtype=np.float32).reshape(K).astype(BF16_NP)
    stat = np.zeros((2 * K, 32 * NSTAT), dtype=BF16_NP)
    for r in range(NSTAT):
        for a in range(2):
            stat[49 * a : 49 * a + 49, 32 * r + 2 * r + a] = wb

    bf = np.ascontiguousarray(np.asarray(bias), dtype=np.float32).reshape(1)
    return xT, stat, bf


def run(enc_x, weight, bias, trace=False, **spmd_kwargs):
    """Run on 8 NeuronCores; returns (out [B, WINDOWS] fp32, BassKernelResults)."""
    from concourse.bass_utils import run_bass_kernel_spmd

    nc = _get_nc()
    xT, stat, bf = _pack_inputs(enc_x, weight, bias)
    in_maps = [{"x": xT[i], "s": stat, "b": bf} for i in range(NCORES)]
    res = run_bass_kernel_spmd(
        nc, in_maps, list(range(NCORES)), trace=trace, **spmd_kwargs
    )
    out = np.stack([res.results[i]["out"] for i in range(NCORES)], axis=0)
    return out.reshape(B, WINDOWS), res


def kernel(enc_x, weight, bias, windows_nb=None):
    out, _ = run(enc_x, weight, bias)
    return out


# revision 11
# speedup vs baseline: 1.0048x; 1.0048x over previous
"""Trainium2 Bass kernel for im2col Conv2d dot-product:
out[b, n] = <enc_x[b, n, :], w_flat> + bias.

Data-parallel over batch: 8 batches per NeuronCore x 8 cores.
Per core: x is [401408, 49] -> out [401408] fp32.

v2: TensorE matmul formulation (baseline v1 was DVE/GpSimd-bound at
344us: tensor_reduce is capped at 1x = ~160us alone, DMA only 61% busy).

  - Host repacks x to bf16, k-major pair layout: for window-row pair
    q (rows 2q, 2q+1 of the [128, 3136] output grid), xT[q] is
    [98, 3136] with partition c = 49*a + k, free j.  Halves HBM
    traffic (39.3 MB/core) and puts the contraction dim on partitions.
  - Stationary: 16 zero-padded block-diagonal weight tiles [98, 32]
    (r = q%16): col 2r+a carries w at rows 49a..49a+48.  A matmul
    lhsT=stat[:,32r:32r+32], rhs=xT[q] chunk writes out rows
    [32b, 32b+32) of PSUM (b = q//16; out partition base 32b is legal:
    32-aligned tile_position).  Multiply+reduce happen in the PE
    array; 448 matmuls x 512 cols ~ 96us, under the ~110us DMA floor.
  - Zero columns of the stationary write zeros; 16 r-matmuls per strip
    accumulate (start at r=0, stop at r=15), each contributing its 2
    real rows.
  - Strip close: ScalarE activation adds bias and copies PSUM->SBUF,
    gpsimd-queue DMA writes the strip; overlaps the next strip's
    matmul stream.  Tail after last matmul is ~4us.
"""

from contextlib import ExitStack

import numpy as np
import ml_dtypes

import concourse.bass as bass
import concourse.tile as tile
from concourse import mybir

B = 64
WINDOWS = 50176
K = 49
NCORES = 8
BPC = B // NCORES            # batches per core
NWIN = BPC * WINDOWS         # 401408 windows per core
ROWS = 128                   # window-row grid: NWIN = ROWS * J
J = NWIN // ROWS             # 3136
NPAIR = ROWS // 2            # 64 row pairs (q)
NSTAT = 16                   # stationaries (r = q % 16)
CHUNK = 512                  # matmul moving cols = one PSUM bank of fp32
G = 4                        # pairs loaded per DMA (16 q's per strip, G | 16)

FP32 = mybir.dt.float32
BF16 = mybir.dt.bfloat16
BF16_NP = ml_dtypes.bfloat16

_NC = None


def _build_nc():
    nc = bass.Bass(trn_type="TRN2", debug=False, num_devices=NCORES)

    x = nc.dram_tensor(
        "x", [NPAIR // G, 2 * K, G * J], BF16, kind="ExternalInput"
    ).ap()
    s = nc.dram_tensor("s", [2 * K, 32 * NSTAT], BF16, kind="ExternalInput").ap()
    b = nc.dram_tensor("b", [1], FP32, kind="ExternalInput").ap()
    out = nc.dram_tensor("out", [NWIN], FP32, kind="ExternalOutput").ap()

    with tile.TileContext(nc) as tc, ExitStack() as ctx:
        consts = ctx.enter_context(tc.tile_pool(name="consts", bufs=1))
        xpool = ctx.enter_context(tc.tile_pool(name="x", bufs=4))
        ppool = ctx.enter_context(tc.tile_pool(name="psum", bufs=1, space="PSUM"))

        stat = consts.tile([2 * K, 32 * NSTAT], BF16)
        nc.gpsimd.dma_start(out=stat[:], in_=s)
        bb = consts.tile([128, 1], FP32)
        nc.gpsimd.dma_start(
            out=bb[:],
            in_=bass.AP(tensor=b.tensor, offset=b.offset, ap=[[0, 128]] + list(b.ap)),
        )

        # Matmul out partition base is restricted to {0, 32, 64} (no 96):
        # psum holds 3 strips; strip 3 reuses psum base 0, whose strip-0
        # accumulation closed 32 pair-tiles earlier (no stall, no hazard).
        acc = ppool.tile([96, J], FP32)
        ot = consts.tile([96, J], FP32)

        # x is group-major: group g's [98, G*J] block is one contiguous
        # 2.46 MB DRAM region (sequential HBM reads, like v1's 375 GB/s
        # tiles), with 25 KB per-partition descriptors.
        for g in range(NPAIR // G):
            xt = xpool.tile([2 * K, G * J], BF16, tag="xt", name=f"xt{g}")
            src = bass.AP(
                tensor=x.tensor,
                offset=x.offset + g * (2 * K) * G * J,
                ap=[[G * J, 2 * K], [1, G * J]],
            )
            nc.sync.dma_start(out=xt[:], in_=src)

            for gq in range(G):
                q = g * G + gq
                r, bstrip = q % NSTAT, q // NSTAT
                p0 = 32 * (bstrip % 3)
                for j0 in range(0, J, CHUNK):
                    j1 = min(J, j0 + CHUNK)
                    nc.tensor.matmul(
                        acc[p0 : p0 + 32, j0:j1],
                        stat[:, 32 * r : 32 * r + 32],
                        xt[:, gq * J + j0 : gq * J + j1],
                        start=(r == 0),
                        stop=(r == NSTAT - 1),
                    )

                if r == NSTAT - 1:
                    # strip complete: bias-add into SBUF, DMA out
                    nc.scalar.activation(
                        out=ot[p0 : p0 + 32, :],
                        in_=acc[p0 : p0 + 32, :],
                        func=mybir.ActivationFunctionType.Identity,
                        bias=bb[p0 : p0 + 32, 0:1],
                        scale=1.0,
                    )
                    dst = bass.AP(
                        tensor=out.tensor,
                        offset=out.offset + 32 * bstrip * J,
                        ap=[[J, 32], [1, J]],
                    )
                    nc.gpsimd.dma_start(out=dst, in_=ot[p0 : p0 + 32, :])

    return nc


def _split_ctrl_waits(nc, max_waits=1):
    """Work around a walrus codegen limit on this build: instructions accept
    only one sync-wait command. Hoist extra waits onto dedicated no-op
    instructions inserted just before, preserving per-engine order."""
    from concourse import mybir

    for f in nc.m.functions:
        for blk in f.blocks:
            insts = blk.instructions
            i = 0
            while i < len(insts):
                ins = insts[i]
                if (
                    ins.sync_info is not None
                    and len(ins.sync_info.on_wait) > max_waits
                ):
                    waits = list(ins.sync_info.on_wait)
                    keep, extra = waits[:max_waits], waits[max_waits:]
                    ins.sync_info.on_wait = keep
                    for j, wchunk in enumerate(extra):
                        nop = mybir.InstNoOp(
                            name=f"{ins.name}-wsplit{j}",
                            sync_info=mybir.SyncInfo(on_wait=[wchunk], on_update=[]),
                            bass_nofuse=True,
                            engine=ins.engine,
                        )
                        nc.register_instruction(nop, overwrite=True)
                        insts.insert(i, nop)
                        i += 1
                i += 1


def _get_nc():
    global _NC
    if _NC is None:
        _NC = _build_nc()
        _split_ctrl_waits(_NC)
    return _NC


def _pack_inputs(enc_x, weight, bias):
    """Host-side repack: bf16 k-major pair layout + stationary tiles."""
    # group-major: xT[c, g, 49*a + k, gq*J + j] = enc_x_core_c[(2q+a)*J + j, k]
    # with q = G*g + gq
    xb = np.asarray(enc_x, dtype=np.float32).reshape(NCORES, NPAIR // G, G, 2, J, K)
    xT = xb.transpose(0, 1, 3, 5, 2, 4).astype(BF16_NP)
    xT = np.ascontiguousarray(xT).reshape(NCORES, NPAIR // G, 2 * K, G * J)

    wb = np.asarray(weight, dtype=np.float32).reshape(K).astype(BF16_NP)
    stat = np.zeros((2 * K, 32 * NSTAT), dtype=BF16_NP)
    for r in range(NSTAT):
        for a in range(2):
            stat[49 * a : 49 * a + 49, 32 * r + 2 * r + a] = wb

    bf = np.ascontiguousarray(np.asarray(bias), dtype=np.float32).reshape(1)
    return xT, stat, bf


def run(enc_x, weight, bias, trace=False, **spmd_kwargs):
    """Run on 8 NeuronCores; returns (out [B, WINDOWS] fp32, BassKernelResults)."""
    from concourse.bass_utils import run_bass_kernel_spmd

    nc = _get_nc()
    xT, stat, bf = _pack_inputs(enc_x, weight, bias)
    in_maps = [{"x": xT[i], "s": stat, "b": bf} for i in range(NCORES)]
    res = run_bass_kernel_spmd(
        nc, in_maps, list(range(NCORES)), trace=trace, **spmd_kwargs
    )
    out = np.stack([res.results[i]["out"] for i in range(NCORES)], axis=0)
    return out.reshape(B, WINDOWS), res


def kernel(enc_x, weight, bias, windows_nb=None):
    out, _ = run(enc_x, weight, bias)
    return out


# revision 12
# speedup vs baseline: 1.0155x; 1.0106x over previous
"""Trainium2 Bass kernel for im2col Conv2d dot-product:
out[b, n] = <enc_x[b, n, :], w_flat> + bias.

Data-parallel over batch: 8 batches per NeuronCore x 8 cores.
Per core: x is [401408, 49] -> out [401408] fp32.

v2: TensorE matmul formulation (baseline v1 was DVE/GpSimd-bound at
344us: tensor_reduce is capped at 1x = ~160us alone, DMA only 61% busy).

  - Host repacks x to bf16, k-major pair layout: for window-row pair
    q (rows 2q, 2q+1 of the [128, 3136] output grid), xT[q] is
    [98, 3136] with partition c = 49*a + k, free j.  Halves HBM
    traffic (39.3 MB/core) and puts the contraction dim on partitions.
  - Stationary: 16 zero-padded block-diagonal weight tiles [98, 32]
    (r = q%16): col 2r+a carries w at rows 49a..49a+48.  A matmul
    lhsT=stat[:,32r:32r+32], rhs=xT[q] chunk writes out rows
    [32b, 32b+32) of PSUM (b = q//16; out partition base 32b is legal:
    32-aligned tile_position).  Multiply+reduce happen in the PE
    array; 448 matmuls x 512 cols ~ 96us, under the ~110us DMA floor.
  - Zero columns of the stationary write zeros; 16 r-matmuls per strip
    accumulate (start at r=0, stop at r=15), each contributing its 2
    real rows.
  - Strip close: ScalarE activation adds bias and copies PSUM->SBUF,
    gpsimd-queue DMA writes the strip; overlaps the next strip's
    matmul stream.  Tail after last matmul is ~4us.
"""

from contextlib import ExitStack

import numpy as np
import ml_dtypes

import concourse.bass as bass
import concourse.tile as tile
from concourse import mybir

B = 64
WINDOWS = 50176
K = 49
NCORES = 8
BPC = B // NCORES            # batches per core
NWIN = BPC * WINDOWS         # 401408 windows per core
ROWS = 128                   # window-row grid: NWIN = ROWS * J
J = NWIN // ROWS             # 3136
NPAIR = ROWS // 2            # 64 row pairs (q)
NSTAT = 16                   # stationaries (r = q % 16)
CHUNK = 512                  # matmul moving cols = one PSUM bank of fp32
G = 4                        # pairs loaded per DMA (16 q's per strip, G | 16)

FP32 = mybir.dt.float32
BF16 = mybir.dt.bfloat16
BF16_NP = ml_dtypes.bfloat16

_NC = None


def _build_nc():
    nc = bass.Bass(trn_type="TRN2", debug=False, num_devices=NCORES)

    x = nc.dram_tensor(
        "x", [NPAIR // G, 2 * K, G * J], BF16, kind="ExternalInput"
    ).ap()
    s = nc.dram_tensor("s", [2 * K, 32 * NSTAT], BF16, kind="ExternalInput").ap()
    b = nc.dram_tensor("b", [1], FP32, kind="ExternalInput").ap()
    out = nc.dram_tensor("out", [NWIN], FP32, kind="ExternalOutput").ap()

    with tile.TileContext(nc) as tc, ExitStack() as ctx:
        consts = ctx.enter_context(tc.tile_pool(name="consts", bufs=1))
        xpool = ctx.enter_context(tc.tile_pool(name="x", bufs=4))
        ppool = ctx.enter_context(tc.tile_pool(name="psum", bufs=1, space="PSUM"))

        stat = consts.tile([2 * K, 32 * NSTAT], BF16)
        nc.gpsimd.dma_start(out=stat[:], in_=s)
        bb = consts.tile([128, 1], FP32)
        nc.gpsimd.dma_start(
            out=bb[:],
            in_=bass.AP(tensor=b.tensor, offset=b.offset, ap=[[0, 128]] + list(b.ap)),
        )

        # Matmul out partition base is restricted to {0, 32, 64} (no 96):
        # psum holds 3 strips; strip 3 reuses psum base 0, whose strip-0
        # accumulation closed 32 pair-tiles earlier (no stall, no hazard).
        acc = ppool.tile([96, J], FP32)
        ot = consts.tile([96, J], FP32)

        # x is group-major: group g's [98, G*J] block is one contiguous
        # 2.46 MB DRAM region (sequential HBM reads, like v1's 375 GB/s
        # tiles), with 25 KB per-partition descriptors.
        for g in range(NPAIR // G):
            xt = xpool.tile([2 * K, G * J], BF16, tag="xt", name=f"xt{g}")
            src = bass.AP(
                tensor=x.tensor,
                offset=x.offset + g * (2 * K) * G * J,
                ap=[[G * J, 2 * K], [1, G * J]],
            )
            # DMA as 4-byte elements (same bytes): 2-byte-element DMAs
            # measured ~16-20 GB/s/engine vs ~27 GB/s for 4-byte.
            nc.sync.dma_start(
                out=xt[:].bitcast(mybir.dt.uint32), in_=src.bitcast(mybir.dt.uint32)
            )

            for gq in range(G):
                q = g * G + gq
                r, bstrip = q % NSTAT, q // NSTAT
                p0 = 32 * (bstrip % 3)
                for j0 in range(0, J, CHUNK):
                    j1 = min(J, j0 + CHUNK)
                    nc.tensor.matmul(
                        acc[p0 : p0 + 32, j0:j1],
                        stat[:, 32 * r : 32 * r + 32],
                        xt[:, gq * J + j0 : gq * J + j1],
                        start=(r == 0),
                        stop=(r == NSTAT - 1),
                    )

                if r == NSTAT - 1:
                    # strip complete: bias-add into SBUF, DMA out
                    nc.scalar.activation(
                        out=ot[p0 : p0 + 32, :],
                        in_=acc[p0 : p0 + 32, :],
                        func=mybir.ActivationFunctionType.Identity,
                        bias=bb[p0 : p0 + 32, 0:1],
                        scale=1.0,
                    )
                    dst = bass.AP(
                        tensor=out.tensor,
                        offset=out.offset + 32 * bstrip * J,
                        ap=[[J, 32], [1, J]],
                    )
                    nc.gpsimd.dma_start(out=dst, in_=ot[p0 : p0 + 32, :])

    return nc


def _split_ctrl_waits(nc, max_waits=1):
    """Work around a walrus codegen limit on this build: instructions accept
    only one sync-wait command. Hoist extra waits onto dedicated no-op
    instructions inserted just before, preserving per-engine order."""
    from concourse import mybir

    for f in nc.m.functions:
        for blk in f.blocks:
            insts = blk.instructions
            i = 0
            while i < len(insts):
                ins = insts[i]
                if (
                    ins.sync_info is not None
                    and len(ins.sync_info.on_wait) > max_waits
                ):
                    waits = list(ins.sync_info.on_wait)
                    keep, extra = waits[:max_waits], waits[max_waits:]
                    ins.sync_info.on_wait = keep
                    for j, wchunk in enumerate(extra):
                        nop = mybir.InstNoOp(
                            name=f"{ins.name}-wsplit{j}",
                            sync_info=mybir.SyncInfo(on_wait=[wchunk], on_update=[]),
                            bass_nofuse=True,
                            engine=ins.engine,
                        )
                        nc.register_instruction(nop, overwrite=True)
                        insts.insert(i, nop)
                        i += 1
                i += 1


def _get_nc():
    global _NC
    if _NC is None:
        _NC = _build_nc()
        _split_ctrl_waits(_NC)
    return _NC


def _pack_inputs(enc_x, weight, bias):
    """Host-side repack: bf16 k-major pair layout + stationary tiles."""
    # group-major: xT[c, g, 49*a + k, gq*J + j] = enc_x_core_c[(2q+a)*J + j, k]
    # with q = G*g + gq
    xb = np.asarray(enc_x, dtype=np.float32).reshape(NCORES, NPAIR // G, G, 2, J, K)
    xT = xb.transpose(0, 1, 3, 5, 2, 4).astype(BF16_NP)
    xT = np.ascontiguousarray(xT).reshape(NCORES, NPAIR // G, 2 * K, G * J)

    wb = np.asarray(weight, dtype=np.float32).reshape(K).astype(BF16_NP)
    stat = np.zeros((2 * K, 32 * NSTAT), dtype=BF16_NP)
    for r in range(NSTAT):
        for a in range(2):
            stat[49 * a : 49 * a + 49, 32 * r + 2 * r + a] = wb

    bf = np.ascontiguousarray(np.asarray(bias), dtype=np.float32).reshape(1)
    return xT, stat, bf


def run(enc_x, weight, bias, trace=False, **spmd_kwargs):
    """Run on 8 NeuronCores; returns (out [B, WINDOWS] fp32, BassKernelResults)."""
    from concourse.bass_utils import run_bass_kernel_spmd

    nc = _get_nc()
    xT, stat, bf = _pack_inputs(enc_x, weight, bias)
    in_maps = [{"x": xT[i], "s": stat, "b": bf} for i in range(NCORES)]
    res = run_bass_kernel_spmd(
        nc, in_maps, list(range(NCORES)), trace=trace, **spmd_kwargs
    )
    out = np.stack([res.results[i]["out"] for i in range(NCORES)], axis=0)
    return out.reshape(B, WINDOWS), res


def kernel(enc_x, weight, bias, windows_nb=None):
    out, _ = run(enc_x, weight, bias)
    return out


# revision 13
# speedup vs baseline: 1.5794x; 1.5554x over previous
"""Trainium2 Bass kernel for im2col Conv2d dot-product:
out[b, n] = <enc_x[b, n, :], w_flat> + bias.

Data-parallel over batch: 8 batches per NeuronCore x 8 cores.
Per core: x is [401408, 49] -> out [401408] fp32.

v5: dense phase-packed TensorE matmul.

DMA on TRN2 runs at full rate (~427 GB/s, 27 GB/s x 16 engines)
ONLY for 128-partition tiles (measured: 98 partitions -> ~260 GB/s,
112 -> ~263, 120 -> ~199). So the moving operand must fill all 128
partitions with real data, but windows are 49 long: instead of
2-windows-per-column (98 rows + 30 pad), pack the im2col stream
DENSELY: column c of "phase" phi holds flat element 128*phi + c of a
6272-element group (= 128 windows x 49 = lcm(49,128) structure).
The identity 128*phi + c = 49*m + k uniquely assigns every (phi, c)
to window-row m, element k, so 49 banded stationaries
S_phi[c, m] = w[128*phi + c - 49*m] (in [0,49)) make the 49
phase-matmuls accumulate exact per-window dot products into
PSUM[m, g] with zero padding and zero redundancy.

Stream order puts window m*3136+g at stream slot 128*g+m, so
PSUM[m, g-chunk] lands in natural window order: out DMA is
contiguous per partition.  Each 448-column block closes after its
49 matmuls -> ScalarE bias-add -> out DMA, fully pipelined (no
end-of-kernel PSUM flush tail).

Per core: DMA in 39.3 MB bf16 (~95-105 us at full rate), TensorE
153664 moving columns ~ 64 us @2.4 GHz, ScalarE 3.6 us, DVE idle.
"""

from contextlib import ExitStack

import numpy as np
import ml_dtypes

import concourse.bass as bass
import concourse.tile as tile
from concourse import mybir

B = 64
WINDOWS = 50176
K = 49
NCORES = 8
BPC = B // NCORES            # batches per core
NWIN = BPC * WINDOWS         # 401408 windows per core
ROWS = 128                   # window-row grid: window = m*J + g
J = NWIN // ROWS             # 3136
NPHI = K                     # 49 phases
GW = 448                     # g-columns per block (PSUM region, 7*GW = J)
NBLK = J // GW               # 7

FP32 = mybir.dt.float32
BF16 = mybir.dt.bfloat16
BF16_NP = ml_dtypes.bfloat16

_NC = None


def _build_nc():
    nc = bass.Bass(trn_type="TRN2", debug=False, num_devices=NCORES)

    # x[blk, c, NPHI*GW]: tile column NPHI*gw? no: column phi*GW+gw
    x = nc.dram_tensor(
        "x", [NBLK, ROWS, NPHI * GW], BF16, kind="ExternalInput"
    ).ap()
    s = nc.dram_tensor("s", [ROWS, NPHI * ROWS], BF16, kind="ExternalInput").ap()
    b = nc.dram_tensor("b", [1], FP32, kind="ExternalInput").ap()
    out = nc.dram_tensor("out", [NWIN], FP32, kind="ExternalOutput").ap()

    with tile.TileContext(nc) as tc, ExitStack() as ctx:
        consts = ctx.enter_context(tc.tile_pool(name="consts", bufs=1))
        xpool = ctx.enter_context(tc.tile_pool(name="x", bufs=3))
        opool = ctx.enter_context(tc.tile_pool(name="o", bufs=2))
        ppool = ctx.enter_context(tc.tile_pool(name="psum", bufs=4, space="PSUM"))

        stat = consts.tile([ROWS, NPHI * ROWS], BF16)
        nc.gpsimd.dma_start(out=stat[:], in_=s)
        bb = consts.tile([ROWS, 1], FP32)
        nc.gpsimd.dma_start(
            out=bb[:],
            in_=bass.AP(tensor=b.tensor, offset=b.offset, ap=[[0, ROWS]] + list(b.ap)),
        )

        for blk in range(NBLK):
            xt = xpool.tile([ROWS, NPHI * GW], BF16, tag="xt", name=f"xt{blk}")
            src = bass.AP(
                tensor=x.tensor,
                offset=x.offset + blk * ROWS * NPHI * GW,
                ap=[[NPHI * GW, ROWS], [1, NPHI * GW]],
            )
            nc.sync.dma_start(out=xt[:], in_=src)

            # allocate a full PSUM bank (512 fp32) for alignment; use GW cols
            acc = ppool.tile([ROWS, 512], FP32, tag="acc", name=f"acc{blk}")
            for phi in range(NPHI):
                nc.tensor.matmul(
                    acc[:, 0:GW],
                    stat[:, phi * ROWS : (phi + 1) * ROWS],
                    xt[:, phi * GW : (phi + 1) * GW],
                    start=(phi == 0),
                    stop=(phi == NPHI - 1),
                )

            ot = opool.tile([ROWS, GW], FP32, tag="ot", name=f"ot{blk}")
            nc.scalar.activation(
                out=ot[:],
                in_=acc[:, 0:GW],
                func=mybir.ActivationFunctionType.Identity,
                bias=bb[:, 0:1],
                scale=1.0,
            )
            dst = bass.AP(
                tensor=out.tensor,
                offset=out.offset + blk * GW,
                ap=[[J, ROWS], [1, GW]],
            )
            nc.gpsimd.dma_start(out=dst, in_=ot[:])

    return nc


def _split_ctrl_waits(nc, max_waits=1):
    """Work around a walrus codegen limit on this build: instructions accept
    only one sync-wait command. Hoist extra waits onto dedicated no-op
    instructions inserted just before, preserving per-engine order."""
    from concourse import mybir

    for f in nc.m.functions:
        for blk in f.blocks:
            insts = blk.instructions
            i = 0
            while i < len(insts):
                ins = insts[i]
                if (
                    ins.sync_info is not None
                    and len(ins.sync_info.on_wait) > max_waits
                ):
                    waits = list(ins.sync_info.on_wait)
                    keep, extra = waits[:max_waits], waits[max_waits:]
                    ins.sync_info.on_wait = keep
                    for j, wchunk in enumerate(extra):
                        nop = mybir.InstNoOp(
                            name=f"{ins.name}-wsplit{j}",
                            sync_info=mybir.SyncInfo(on_wait=[wchunk], on_update=[]),
                            bass_nofuse=True,
                            engine=ins.engine,
                        )
                        nc.register_instruction(nop, overwrite=True)
                        insts.insert(i, nop)
                        i += 1
                i += 1


def _get_nc():
    global _NC
    if _NC is None:
        _NC = _build_nc()
        _split_ctrl_waits(_NC)
    return _NC


# z = 128*phi + c = 49*m + k for z in [0, 6272)
_Z = np.arange(ROWS * K)
_MZ = _Z // K
_KZ = _Z % K


def _pack_inputs(enc_x, weight, bias):
    """Host-side repack: dense phase-packed bf16 layout + banded stationaries."""
    # xr[m, g, k] = enc_x_core[m*J + g, k]; phase view:
    # X4[phi, c, g] = xr[mz, g, kz] at z = 128*phi + c
    xr = np.asarray(enc_x, dtype=np.float32).reshape(NCORES, ROWS, J, K)
    xb = xr.astype(BF16_NP)
    xT = np.empty((NCORES, NBLK, ROWS, NPHI * GW), dtype=BF16_NP)
    for cix in range(NCORES):
        g1 = xb[cix][_MZ, :, _KZ]                      # [6272, J]
        x4 = g1.reshape(NPHI, ROWS, NBLK, GW)          # [phi, c, blk, gw]
        xT[cix] = x4.transpose(2, 1, 0, 3).reshape(NBLK, ROWS, NPHI * GW)

    wb = np.asarray(weight, dtype=np.float32).reshape(K).astype(BF16_NP)
    stat = np.zeros((ROWS, NPHI * ROWS), dtype=BF16_NP)
    for phi in range(NPHI):
        z = 128 * phi + np.arange(ROWS)                # z for each c
        m = z // K
        k = z % K
        valid = m < ROWS
        stat[np.arange(ROWS)[valid], phi * ROWS + m[valid]] = wb[k[valid]]

    bf = np.ascontiguousarray(np.asarray(bias), dtype=np.float32).reshape(1)
    return xT, stat, bf


def run(enc_x, weight, bias, trace=False, **spmd_kwargs):
    """Run on 8 NeuronCores; returns (out [B, WINDOWS] fp32, BassKernelResults)."""
    from concourse.bass_utils import run_bass_kernel_spmd

    nc = _get_nc()
    xT, stat, bf = _pack_inputs(enc_x, weight, bias)
    in_maps = [{"x": xT[i], "s": stat, "b": bf} for i in range(NCORES)]
    res = run_bass_kernel_spmd(
        nc, in_maps, list(range(NCORES)), trace=trace, **spmd_kwargs
    )
    out = np.stack([res.results[i]["out"] for i in range(NCORES)], axis=0)
    return out.reshape(B, WINDOWS), res


def kernel(enc_x, weight, bias, windows_nb=None):
    out, _ = run(enc_x, weight, bias)
    return out
